# revision 1
# baseline (speedup 1.0000x reference)
"""HNHN hypergraph GNN forward on 8 Trainium2 NeuronCores (Bass/Tile).

Sharding: edges 50k/core, nodes 25k/core (edge ids relabeled e -> (e%8, e//8)
for load balance; relabeling is internal, the output is a node max-pool).
Each segment aggregation is computed as PE matmuls
    psum[feat, 512segs] += G_block^T @ S_block
with G_block = 128 gathered bf16 source rows and S_block a one-hot x weight
selection matrix built on DVE via tensor_scalar(is_equal, mult) against an
iota tile. Layer-1 edge aggregation consumes a host-expanded per-nnz stream
of x_0 (input resharding; no gather). The other three aggregations gather
device-computed bf16 tables with dma_gather (1024 rows/instruction, int16
indices => 32k-row buckets; nnz sorted by (psum-section, bucket, seg); runs
padded with trailing -1 indices which the DMA skips). The per-layer dense
matmul, sigmoid (+per-partition bias) happen in the transposed [feat, seg]
domain; PE transposes restore row-major bf16 tables which are AllGather'd
for the next aggregation. Final: running window max, AllReduce(max), f32 dot
with lin_w.
"""

import numpy as np
import ml_dtypes

bf16 = ml_dtypes.bfloat16
f32 = np.float32

P = 128
WIN = 512          # segments per PSUM window (one bank)
SECW = 5           # windows per section (PSUM: 5 win + 1 tp + 2 m = 8 banks)
BUCK = 32768       # dma_gather int16 index range
NIMAX = 1024       # dma_gather max idxs/instruction (ring capacity)
NCORES = 8

N_NODES = 200_000
N_EDGES = 400_000
IN_CH = 14
HID = 128
ALPHA = -1.5
BETA = -0.5


def _dims():
    node_loc = N_NODES // NCORES
    edge_loc = N_EDGES // NCORES
    node_wins = -(-node_loc // WIN)
    edge_wins = -(-edge_loc // WIN)
    return dict(
        node_loc=node_loc, edge_loc=edge_loc,
        node_pad=node_wins * WIN, edge_pad=edge_wins * WIN,
        node_tab=node_wins * WIN * NCORES, edge_tab=edge_wins * WIN * NCORES,
    )


def _wrap_idx(flat):
    a = flat.reshape(-1, 16).T.astype(np.int16)
    return np.tile(a, (8, 1))


class AggSched:
    """Schedule + per-core metadata for one aggregation (SPMD-identical)."""

    def __init__(self, name, dest_loc, src_all, w_all, n_seg_loc, tab_rows,
                 gathered):
        self.name = name
        self.gathered = gathered
        self.n_seg_loc = n_seg_loc
        self.n_wins = -(-n_seg_loc // WIN)
        sec = WIN * SECW
        self.n_secs = -(-self.n_wins // SECW)
        nbuck = -(-tab_rows // BUCK) if gathered else 1

        per_core = []
        counts = np.zeros((NCORES, self.n_secs, nbuck), np.int64)
        for r in range(NCORES):
            d = dest_loc[r].astype(np.int64)
            s = src_all[r].astype(np.int64)
            w = w_all[r].astype(f32)
            sc = d // sec
            b = (s // BUCK) if gathered else np.zeros_like(s)
            order = np.lexsort((d, b, sc))
            d, s, w, sc, b = d[order], s[order], w[order], sc[order], b[order]
            per_core.append((d, s, w, sc, b))
            np.add.at(counts[r], (sc, b), 1)

        self.runs = []          # (sec, bucket, n_pad_slots)
        for sc in range(self.n_secs):
            for b in range(nbuck):
                c = counts[:, sc, b].max()
                if c:
                    self.runs.append((sc, b, int(-(-c // 128) * 128)))
        total_slots = sum(np_ for _, _, np_ in self.runs)
        self.n_blocks = total_slots // 128

        dmat = np.full((NCORES, total_slots), -1, np.int64)
        smat = np.full((NCORES, total_slots), -1, np.int64)
        wmat = np.zeros((NCORES, total_slots), f32)
        for r in range(NCORES):
            d, s, w, sc, b = per_core[r]
            off = 0
            ptr = 0
            for rsec, rb, n_pad in self.runs:
                cnt = int(counts[r, rsec, rb])
                dmat[r, off:off + cnt] = d[ptr:ptr + cnt]
                smat[r, off:off + cnt] = (s[ptr:ptr + cnt] % BUCK) if gathered \
                    else s[ptr:ptr + cnt]
                if gathered:
                    smat[r, off + cnt:off + n_pad] = 0  # pad -> bucket row 0
                wmat[r, off:off + cnt] = w[ptr:ptr + cnt]
                ptr += cnt
                off += n_pad
            assert ptr == len(d)

        self.insts = []         # (bucket, slot_off, ni)
        off = 0
        for sc, b, n_pad in self.runs:
            o = 0
            while o < n_pad:
                ni = min(NIMAX, n_pad - o)
                self.insts.append((b, off + o, ni))
                o += ni
            off += n_pad

        dblk = dmat.reshape(NCORES, self.n_blocks, 128)
        self.jobs = []          # (block, window, c0, c1)
        for blk in range(self.n_blocks):
            dv = dblk[:, blk, :]
            valid = dv >= 0
            if not valid.any():
                continue
            for wname in np.unique(dv[valid] // WIN):
                m = valid & (dv // WIN == wname)
                c0 = int((dv[m] - wname * WIN).min())
                c1 = int((dv[m] - wname * WIN).max()) + 1
                self.jobs.append((blk, int(wname), c0, c1))
        self.n_jobs = len(self.jobs)
        self.win_last = {}
        for j, (blk, wname, c0, c1) in enumerate(self.jobs):
            self.win_last[wname] = j

        self.w_arr = np.ascontiguousarray(
            wmat.reshape(NCORES, self.n_blocks, 128).transpose(0, 2, 1))
        self.seg_arr = np.full((NCORES, P, max(self.n_jobs, 1)), -1.0, f32)
        for j, (blk, wname, c0, c1) in enumerate(self.jobs):
            dv = dblk[:, blk, :]                    # [NCORES, 128]
            ok = (dv >= 0) & (dv // WIN == wname)
            self.seg_arr[:, :, j] = np.where(ok, dv - wname * WIN, -1.0)
        self.seg_arr = self.seg_arr.astype(f32)

        if gathered:
            idx_cols = sum(ni for _, _, ni in self.insts) // 16
            self.idx_arr = np.zeros((NCORES, P, idx_cols), np.int16)
            self.inst_idx_off = []
            col = 0
            for b, slot_off, ni in self.insts:
                self.inst_idx_off.append(col)
                for r in range(NCORES):
                    self.idx_arr[r, :, col:col + ni // 16] = _wrap_idx(
                        smat[r, slot_off:slot_off + ni].astype(np.int16))
                col += ni // 16
            self.idx_cols = idx_cols
        else:
            self.smat = smat


def _preprocess(inputs):
    dims = _dims()
    rows = np.asarray(inputs["inc_rows"]).astype(np.int64)
    cols0 = np.asarray(inputs["inc_cols"]).astype(np.int64)
    vals = np.asarray(inputs["inc_vals"]).astype(f32)

    # relabel edges for per-core balance: e -> (e % NCORES)*edge_loc + e//NCORES
    cols = (cols0 % NCORES) * dims["edge_loc"] + cols0 // NCORES

    deg_e = np.bincount(cols, weights=vals, minlength=N_EDGES).astype(f32)
    deg_v = np.bincount(rows, weights=vals, minlength=N_NODES).astype(f32)
    e_card = deg_e ** f32(ALPHA)
    n_card = deg_v ** f32(BETA)
    denom_v = np.bincount(rows, weights=(vals * e_card[cols]).astype(np.float64),
                          minlength=N_NODES).astype(f32)
    denom_e = np.bincount(cols, weights=(vals * n_card[rows]).astype(np.float64),
                          minlength=N_EDGES).astype(f32)
    w_ev = vals * n_card[rows] / denom_e[cols]
    w_ve = vals * e_card[cols] / denom_v[rows]

    e_core = cols // dims["edge_loc"]
    v_core = rows // dims["node_loc"]
    node_pad_row = rows // dims["node_loc"] * dims["node_pad"] \
        + rows % dims["node_loc"]
    edge_pad_row = cols // dims["edge_loc"] * dims["edge_pad"] \
        + cols % dims["edge_loc"]

    def split(arr, by):
        return [arr[by == r] for r in range(NCORES)]

    e_d = split(cols % dims["edge_loc"], e_core)
    e_s_raw = split(rows, e_core)
    e_s_pad = split(node_pad_row, e_core)
    e_w = split(w_ev, e_core)
    n_d = split(rows % dims["node_loc"], v_core)
    n_s = split(edge_pad_row, v_core)
    n_w = split(w_ve, v_core)

    sched_e1 = AggSched("e1", e_d, e_s_raw, e_w, dims["edge_loc"], N_NODES,
                        False)
    sched_e2 = AggSched("e2", e_d, e_s_pad, e_w, dims["edge_loc"],
                        dims["node_tab"], True)
    sched_n = AggSched("n", n_d, n_s, n_w, dims["node_loc"],
                       dims["edge_tab"], True)

    x0 = np.asarray(inputs["x_0"]).astype(f32)
    x0p = np.zeros((N_NODES + 1, 16), f32)
    x0p[:N_NODES, :IN_CH] = x0
    e1_stream = np.zeros((NCORES, P, sched_e1.n_blocks * 16), bf16)
    for r in range(NCORES):
        src = sched_e1.smat[r].reshape(sched_e1.n_blocks, 128)
        g = x0p[np.where(src >= 0, src, N_NODES)]
        e1_stream[r] = g.transpose(1, 0, 2).reshape(P, -1).astype(bf16)

    return dict(sched_e1=sched_e1, sched_e2=sched_e2, sched_n=sched_n,
                e1_stream=e1_stream, dims=dims)


def _build(pre):
    import concourse.bacc as bacc
    import concourse.mybir as mybir
    import concourse.tile as tile

    dt = mybir.dt
    dims = pre["dims"]
    nc = bacc.Bacc("TRN2", target_bir_lowering=False, debug=False,
                   num_devices=NCORES)

    s_e1, s_e2, s_n = pre["sched_e1"], pre["sched_e2"], pre["sched_n"]

    def din(name, shape, dtyp):
        return nc.dram_tensor(name, shape, dtyp, kind="ExternalInput")

    e1_g = din("e1_g", [P, s_e1.n_blocks * 16], dt.bfloat16)
    e1_seg = din("e1_seg", [P, max(s_e1.n_jobs, 1)], dt.float32)
    e1_w = din("e1_w", [P, s_e1.n_blocks], dt.float32)
    n1_idx = din("n1_idx", [P, s_n.idx_cols], dt.int16)
    n1_seg = din("n1_seg", [P, max(s_n.n_jobs, 1)], dt.float32)
    n1_w = din("n1_w", [P, s_n.n_blocks], dt.float32)
    e2_idx = din("e2_idx", [P, s_e2.idx_cols], dt.int16)
    e2_seg = din("e2_seg", [P, max(s_e2.n_jobs, 1)], dt.float32)
    e2_w = din("e2_w", [P, s_e2.n_blocks], dt.float32)

    w_in = {k: din(k, [kd, HID], dt.bfloat16)
            for k, kd in (("w0_1", 16), ("w1_1", HID), ("w0_2", HID),
                          ("w1_2", HID))}
    b_in = {k: din(k, [P, 1], dt.float32)
            for k in ("b1_1", "b0_1", "b1_2", "b0_2")}
    lin_w = din("lin_w", [P, 1], dt.float32)
    lin_b = din("lin_b", [1, 1], dt.float32)
    iota_in = din("iota", [P, WIN], dt.float32)
    ident_in = din("ident", [P, P], dt.bfloat16)
    out_t = nc.dram_tensor("out", [1, 1], dt.float32, kind="ExternalOutput")

    def dint(name, shape, shared=False):
        return nc.dram_tensor(name, shape, dt.bfloat16, kind="Internal",
                              addr_space="Shared" if shared else "Local")

    x1l1_loc = dint("x1l1_loc", [dims["edge_pad"], HID])
    x1l1_full = dint("x1l1_full", [dims["edge_tab"], HID], True)
    x0p_loc = dint("x0p_loc", [dims["node_pad"], HID])
    x0p_full = dint("x0p_full", [dims["node_tab"], HID], True)
    x1l2_loc = dint("x1l2_loc", [dims["edge_pad"], HID])
    x1l2_full = dint("x1l2_full", [dims["edge_tab"], HID], True)
    armax_in = nc.dram_tensor("armax_in", [P, 1], dt.float32, kind="Internal")
    armax_out = nc.dram_tensor("armax_out", [P, 1], dt.float32,
                               kind="Internal", addr_space="Shared")

    with tile.TileContext(nc) as tc:
        with tc.tile_pool(name="const", bufs=1) as cp, \
             tc.tile_pool(name="meta", bufs=2) as mp, \
             tc.tile_pool(name="gt", bufs=6) as gp, \
             tc.tile_pool(name="st", bufs=4) as sp, \
             tc.tile_pool(name="fl", bufs=2) as fp, \
             tc.tile_pool(name="psw", bufs=1, space="PSUM") as pw, \
             tc.tile_pool(name="psm", bufs=2, space="PSUM") as pm:

            iota_t = cp.tile([P, WIN], dt.float32)
            ident_t = cp.tile([P, P], dt.bfloat16)
            nc.sync.dma_start(iota_t[:], iota_in[:])
            nc.sync.dma_start(ident_t[:], ident_in[:])
            wts, bias = {}, {}
            for k, hnd in w_in.items():
                t = cp.tile(list(hnd.shape), dt.bfloat16, tag=k)
                nc.sync.dma_start(t[:], hnd[:])
                wts[k] = t
            for k, hnd in b_in.items():
                t = cp.tile([P, 1], dt.float32, tag=k)
                nc.sync.dma_start(t[:], hnd[:])
                bias[k] = t
            linw_t = cp.tile([P, 1], dt.float32)
            nc.sync.dma_start(linw_t[:], lin_w[:])
            linb_t = cp.tile([1, 1], dt.float32)
            nc.sync.dma_start(linb_t[:], lin_b[:])
            maxacc = cp.tile([P, WIN], dt.bfloat16)
            nc.vector.memset(maxacc[:], -1.0)

            def run_agg(sched, seg_d, w_d, kdim, wkey, bkey, table, out_loc,
                        idx_d=None, stream_d=None, maxpool=False):
                seg_t = mp.tile([P, max(sched.n_jobs, 1)], dt.float32,
                                tag="seg")
                w_t = mp.tile([P, sched.n_blocks], dt.float32, tag="w")
                nc.sync.dma_start(seg_t[:], seg_d[:])
                nc.sync.dma_start(w_t[:], w_d[:])

                blk_slice = {}
                if sched.gathered:
                    idx_t = mp.tile([P, sched.idx_cols], dt.int16, tag="idx")
                    nc.sync.dma_start(idx_t[:], idx_d[:])
                    tab_rows = table.shape[0]
                    for k, (b, slot_off, ni) in enumerate(sched.insts):
                        g = gp.tile([P, (NIMAX // P) * HID], dt.bfloat16,
                                    tag="g")
                        off = sched.inst_idx_off[k]
                        nc.gpsimd.dma_gather(
                            g[:, :(ni // P) * HID].rearrange(
                                "p (n f) -> p n f", f=HID),
                            table[b * BUCK:min((b + 1) * BUCK, tab_rows), :],
                            idx_t[:, off:off + ni // 16],
                            ni, ni, HID)
                        for cb in range(ni // P):
                            blk_slice[slot_off // P + cb] = (g, cb * HID, HID)
                else:
                    SLAB = 32
                    for sl in range(-(-sched.n_blocks // SLAB)):
                        b0 = sl * SLAB
                        nb = min(SLAB, sched.n_blocks - b0)
                        g = gp.tile([P, SLAB * 16], dt.bfloat16, tag="g")
                        nc.sync.dma_start(g[:, :nb * 16],
                                          stream_d[:, b0 * 16:(b0 + nb) * 16])
                        for cb in range(nb):
                            blk_slice[b0 + cb] = (g, cb * 16, 16)

                win_tiles = {}

                def flush(wn):
                    psum1 = win_tiles.pop(wn)
                    aggt = fp.tile([kdim, WIN], dt.bfloat16, tag="aggt")
                    nc.vector.tensor_copy(aggt[:], psum1[:])
                    psum2 = pm.tile([P, WIN], dt.float32, tag="m",
                                    space="PSUM")
                    nc.tensor.matmul(psum2[:], lhsT=wts[wkey][:], rhs=aggt[:],
                                     start=True, stop=True)
                    xt = fp.tile([P, WIN], dt.bfloat16, tag="xt")
                    nc.scalar.activation(xt[:], psum2[:],
                                         mybir.ActivationFunctionType.Sigmoid,
                                         bias=bias[bkey][:, :1], scale=1.0)
                    if maxpool:
                        nv = min(WIN, sched.n_seg_loc - wn * WIN)
                        nc.vector.tensor_tensor(
                            out=maxacc[:, :nv], in0=maxacc[:, :nv],
                            in1=xt[:, :nv], op=mybir.AluOpType.max)
                    else:
                        nq = WIN // P
                        rowt = fp.tile([P, WIN], dt.bfloat16, tag="rowt")
                        for q in range(nq):
                            pt = pw.tile([P, P], dt.bfloat16, tag="tp",
                                         space="PSUM")
                            nc.tensor.transpose(pt[:],
                                                xt[:, q * P:(q + 1) * P],
                                                ident_t[:])
                            nc.vector.tensor_copy(rowt[:, q * P:(q + 1) * P],
                                                  pt[:])
                        nc.sync.dma_start(
                            out_loc[wn * WIN:(wn + 1) * WIN, :].rearrange(
                                "(q p) f -> p q f", p=P),
                            rowt[:].rearrange("p (q f) -> p q f", q=nq))

                for j, (blk, wn, c0, c1) in enumerate(sched.jobs):
                    g, goff, gw = blk_slice[blk]
                    span = c1 - c0
                    s_t = sp.tile([P, WIN], dt.bfloat16, tag="s")
                    nc.vector.tensor_scalar(
                        out=s_t[:, :span], in0=iota_t[:, c0:c1],
                        scalar1=seg_t[:, j:j + 1],
                        scalar2=w_t[:, blk:blk + 1],
                        op0=mybir.AluOpType.is_equal,
                        op1=mybir.AluOpType.mult)
                    if wn not in win_tiles:
                        pt = pw.tile([kdim, WIN], dt.float32,
                                     tag=f"win{wn % SECW}", space="PSUM")
                        nc.vector.memset(pt[:], 0.0)
                        win_tiles[wn] = pt
                    nc.tensor.matmul(
                        win_tiles[wn][:, c0:c1], lhsT=g[:, goff:goff + gw],
                        rhs=s_t[:, :span], start=False,
                        stop=(sched.win_last[wn] == j),
                        skip_group_check=True)
                    if sched.win_last[wn] == j:
                        flush(wn)

            import os
            PH = int(os.environ.get("PHASES", "4"))
            rg = [list(range(NCORES))]
            run_agg(s_e1, e1_seg, e1_w, 16, "w0_1", "b1_1", None, x1l1_loc,
                    stream_d=e1_g)
            if PH >= 2:
                nc.gpsimd.collective_compute(
                    "AllGather", mybir.AluOpType.bypass, replica_groups=rg,
                    ins=[x1l1_loc[:]], outs=[x1l1_full[:]])
                run_agg(s_n, n1_seg, n1_w, HID, "w1_1", "b0_1", x1l1_full,
                        x0p_loc, idx_d=n1_idx)
            if PH >= 3:
                nc.gpsimd.collective_compute(
                    "AllGather", mybir.AluOpType.bypass, replica_groups=rg,
                    ins=[x0p_loc[:]], outs=[x0p_full[:]])
                run_agg(s_e2, e2_seg, e2_w, HID, "w0_2", "b1_2", x0p_full,
                        x1l2_loc, idx_d=e2_idx)
            if PH >= 4:
                nc.gpsimd.collective_compute(
                    "AllGather", mybir.AluOpType.bypass, replica_groups=rg,
                    ins=[x1l2_loc[:]], outs=[x1l2_full[:]])
                run_agg(s_n, n1_seg, n1_w, HID, "w1_2", "b0_2", x1l2_full,
                        None, idx_d=n1_idx, maxpool=True)

            mx = fp.tile([P, 1], dt.float32, tag="mx")
            nc.vector.reduce_max(out=mx[:], in_=maxacc[:],
                                 axis=mybir.AxisListType.X)
            nc.sync.dma_start(armax_in[:], mx[:])
            nc.gpsimd.collective_compute(
                "AllReduce", mybir.AluOpType.max, replica_groups=rg,
                ins=[armax_in[:]], outs=[armax_out[:]])
            mx2 = fp.tile([P, 1], dt.float32, tag="mx2")
            nc.sync.dma_start(mx2[:], armax_out[:])
            prod = fp.tile([P, 1], dt.float32, tag="prod")
            nc.vector.tensor_mul(prod[:], mx2[:], linw_t[:])
            ones = cp.tile([P, 1], dt.float32, tag="ones")
            nc.vector.memset(ones[:], 1.0)
            psf = pw.tile([1, 1], dt.float32, tag="tp", space="PSUM")
            nc.tensor.matmul(psf[:], lhsT=prod[:], rhs=ones[:],
                             start=True, stop=True)
            res = fp.tile([1, 1], dt.float32, tag="res")
            nc.scalar.activation(res[:], psf[:],
                                 mybir.ActivationFunctionType.Identity,
                                 bias=linb_t[:, :1], scale=1.0)
            nc.sync.dma_start(out_t[:], res[:])

    nc.compile()
    return nc


def make_in_maps(pre, inputs):
    s_e1, s_e2, s_n = pre["sched_e1"], pre["sched_e2"], pre["sched_n"]
    iota = np.broadcast_to(np.arange(WIN, dtype=f32), (P, WIN)).copy()
    ident = np.eye(P, dtype=bf16)

    def b_t(x):
        return np.asarray(x).astype(f32).reshape(HID, 1)

    w0_1 = np.zeros((16, HID), bf16)
    w0_1[:IN_CH] = np.asarray(inputs["w0_l1"]).astype(bf16)
    in_maps = []
    for r in range(NCORES):
        in_maps.append(dict(
            e1_g=pre["e1_stream"][r],
            e1_seg=np.ascontiguousarray(s_e1.seg_arr[r]),
            e1_w=np.ascontiguousarray(s_e1.w_arr[r]),
            n1_idx=np.ascontiguousarray(s_n.idx_arr[r]),
            n1_seg=np.ascontiguousarray(s_n.seg_arr[r]),
            n1_w=np.ascontiguousarray(s_n.w_arr[r]),
            e2_idx=np.ascontiguousarray(s_e2.idx_arr[r]),
            e2_seg=np.ascontiguousarray(s_e2.seg_arr[r]),
            e2_w=np.ascontiguousarray(s_e2.w_arr[r]),
            w0_1=w0_1,
            w1_1=np.asarray(inputs["w1_l1"]).astype(bf16),
            w0_2=np.asarray(inputs["w0_l2"]).astype(bf16),
            w1_2=np.asarray(inputs["w1_l2"]).astype(bf16),
            b1_1=b_t(inputs["b1_l1"]), b0_1=b_t(inputs["b0_l1"]),
            b1_2=b_t(inputs["b1_l2"]), b0_2=b_t(inputs["b0_l2"]),
            lin_w=np.asarray(inputs["lin_w"]).astype(f32).reshape(HID, 1),
            lin_b=np.asarray(inputs["lin_b"]).astype(f32).reshape(1, 1),
            iota=iota, ident=ident,
        ))
    return in_maps


def kernel(**inputs):
    pre = _preprocess(inputs)
    nc = _build(pre)
    in_maps = make_in_maps(pre, inputs)
    from concourse.bass_utils import run_bass_kernel_spmd
    res = run_bass_kernel_spmd(nc, in_maps, core_ids=list(range(NCORES)))
    out = res.results[0]["out"].reshape(1).astype(f32)
    return out



# revision 2
# speedup vs baseline: 85.0326x; 85.0326x over previous
"""HNHN hypergraph GNN forward on 8 Trainium2 NeuronCores (Bass/Tile).

Compact-program design: each of the 4 segment aggregations is a For_i
hardware loop over destination windows (512 segments / PSUM bank). Every
window owns a fixed number of 128-slot blocks (padded; pad slots gather row 0
with weight 0). Per block: one indirect DMA gathers 128 source rows (int32
row ids, one per partition), DVE builds a one-hot x weight selection matrix
S[128,512] via tensor_scalar(is_equal, mult) against an iota tile, PE
accumulates psum[feat,512] += G^T @ S. Window flush: dense weight matmul,
sigmoid(+bias), PE transposes to a row-major bf16 table (window rows stored
p-major: row = w*512 + (c%128)*4 + c//128), AllGather'd for the next
aggregation. Layer-1/2 aggregations share gather metadata (same schedule,
different tables). Final: running window max, AllReduce(max), dot with lin_w.
"""

import os
import numpy as np
import ml_dtypes

bf16 = ml_dtypes.bfloat16
f32 = np.float32

P = 128
WIN = 512
NCORES = 8

N_NODES = 200_000
N_EDGES = 400_000
IN_CH = 14
HID = 128
ALPHA = -1.5
BETA = -0.5

NODE_LOC = N_NODES // NCORES          # 25000
EDGE_LOC = N_EDGES // NCORES          # 50000
NODE_WINS = -(-NODE_LOC // WIN)       # 49
EDGE_WINS = -(-EDGE_LOC // WIN)       # 98
NODE_PAD = NODE_WINS * WIN            # 25088
EDGE_PAD = EDGE_WINS * WIN            # 50176
NODE_TAB = NODE_PAD * NCORES          # 200704
EDGE_TAB = EDGE_PAD * NCORES          # 401408


def _permrow(d):
    """Window-local storage permutation: dest local id -> table row offset."""
    return (d // WIN) * WIN + (d % P) * 4 + (d % WIN) // P


class FamilySched:
    """Window-major padded slot schedule for one destination family."""

    def __init__(self, dest_loc_list, src_row_list, w_list, n_wins, split):
        # per-window nnz counts across cores
        counts = np.zeros((NCORES, n_wins), np.int64)
        for r in range(NCORES):
            win = dest_loc_list[r] // WIN
            np.add.at(counts[r], win, 1)
        # ranges: [(w0, w1, wb, blkbase)]
        self.ranges = []
        blkbase_of_win = np.zeros(n_wins, np.int64)
        base = 0
        bounds = [0] + list(split) + [n_wins]
        for a, b in zip(bounds[:-1], bounds[1:]):
            wb = int(-(-counts[:, a:b].max() // P))
            self.ranges.append((a, b, wb, base))
            for w in range(a, b):
                blkbase_of_win[w] = base + (w - a) * wb
            base += (b - a) * wb
        self.n_blocks = int(base)

        idx = np.zeros((NCORES, P, self.n_blocks), np.int32)
        seg = np.zeros((NCORES, P, self.n_blocks), np.int16)
        wgt = np.zeros((NCORES, P, self.n_blocks), bf16)
        for r in range(NCORES):
            d = dest_loc_list[r]
            s = src_row_list[r]
            w = w_list[r]
            win = d // WIN
            order = np.argsort(win, kind="stable")
            d, s, w, win = d[order], s[order], w[order], win[order]
            # rank within window
            start = np.zeros(n_wins + 1, np.int64)
            np.add.at(start, win + 1, 1)
            start = np.cumsum(start)
            rank = np.arange(len(d)) - start[win]
            col = blkbase_of_win[win] + rank // P
            part = rank % P
            idx[r, part, col] = s
            seg[r, part, col] = d % WIN
            wgt[r, part, col] = w
        self.idx, self.seg, self.wgt = idx, seg, wgt


def _preprocess(inputs):
    rows = np.asarray(inputs["inc_rows"]).astype(np.int64)
    cols0 = np.asarray(inputs["inc_cols"]).astype(np.int64)
    vals = np.asarray(inputs["inc_vals"]).astype(f32)

    # relabel edges for per-core balance: e -> (e % 8)*EDGE_LOC + e//8
    cols = (cols0 % NCORES) * EDGE_LOC + cols0 // NCORES

    deg_e = np.bincount(cols, weights=vals, minlength=N_EDGES).astype(f32)
    deg_v = np.bincount(rows, weights=vals, minlength=N_NODES).astype(f32)
    e_card = deg_e ** f32(ALPHA)
    n_card = deg_v ** f32(BETA)
    denom_v = np.bincount(rows, weights=(vals * e_card[cols]).astype(np.float64),
                          minlength=N_NODES).astype(f32)
    denom_e = np.bincount(cols, weights=(vals * n_card[rows]).astype(np.float64),
                          minlength=N_EDGES).astype(f32)
    w_ev = vals * n_card[rows] / denom_e[cols]
    w_ve = vals * e_card[cols] / denom_v[rows]

    r_e, l_e = cols // EDGE_LOC, cols % EDGE_LOC
    r_v, v_l = rows // NODE_LOC, rows % NODE_LOC
    node_srow = (r_v * NODE_PAD + _permrow(v_l)).astype(np.int32)
    edge_srow = (r_e * EDGE_PAD + _permrow(l_e)).astype(np.int32)

    def split_by(dest_core, *arrs):
        out = []
        for r in range(NCORES):
            m = dest_core == r
            out.append(tuple(a[m] for a in arrs))
        return out

    eparts = split_by(r_e, l_e, node_srow, w_ev)
    nparts = split_by(r_v, v_l, edge_srow, w_ve)

    sched_e = FamilySched([p[0] for p in eparts], [p[1] for p in eparts],
                          [p[2] for p in eparts], EDGE_WINS, split=(49,))
    sched_n = FamilySched([p[0] for p in nparts], [p[1] for p in nparts],
                          [p[2] for p in nparts], NODE_WINS, split=())

    # x0 table, permuted rows, padded to 16 channels, bf16
    x0 = np.asarray(inputs["x_0"]).astype(f32)
    x0tab = np.zeros((NCORES, NODE_PAD, 16), bf16)
    allv = np.arange(N_NODES)
    x0tab[allv // NODE_LOC, _permrow(allv % NODE_LOC)] = \
        np.pad(x0, ((0, 0), (0, 2))).astype(bf16)

    return dict(sched_e=sched_e, sched_n=sched_n, x0tab=x0tab)


def _build(pre):
    import concourse.bacc as bacc
    import concourse.mybir as mybir
    import concourse.tile as tile
    from concourse.bass import ds, IndirectOffsetOnAxis

    dt = mybir.dt
    s_e, s_n = pre["sched_e"], pre["sched_n"]
    nc = bacc.Bacc("TRN2", target_bir_lowering=False, debug=False,
                   num_devices=NCORES)

    def din(name, shape, dtyp):
        return nc.dram_tensor(name, shape, dtyp, kind="ExternalInput")

    x0_in = din("x0_in", [NODE_PAD, 16], dt.bfloat16)
    e_idx = din("e_idx", [P, s_e.n_blocks], dt.int32)
    e_seg = din("e_seg", [P, s_e.n_blocks], dt.int16)
    e_wgt = din("e_wgt", [P, s_e.n_blocks], dt.bfloat16)
    n_idx = din("n_idx", [P, s_n.n_blocks], dt.int32)
    n_seg = din("n_seg", [P, s_n.n_blocks], dt.int16)
    n_wgt = din("n_wgt", [P, s_n.n_blocks], dt.bfloat16)

    w_in = {k: din(k, [kd, HID], dt.bfloat16)
            for k, kd in (("w0_1", 16), ("w1_1", HID), ("w0_2", HID),
                          ("w1_2", HID))}
    b_in = {k: din(k, [P, 1], dt.float32)
            for k in ("b1_1", "b0_1", "b1_2", "b0_2")}
    lin_w = din("lin_w", [P, 1], dt.float32)
    lin_b = din("lin_b", [1, 1], dt.float32)
    iota_in = din("iota", [P, WIN], dt.float32)
    ident_in = din("ident", [P, P], dt.bfloat16)
    out_t = nc.dram_tensor("out", [1, 1], dt.float32, kind="ExternalOutput")

    def dint(name, shape, shared=False):
        return nc.dram_tensor(name, shape, dt.bfloat16, kind="Internal",
                              addr_space="Shared" if shared else "Local")

    x0_loc = dint("x0_loc", [NODE_PAD, 16])
    x0_full = dint("x0_full", [NODE_TAB, 16], True)
    x1l1_loc = dint("x1l1_loc", [EDGE_PAD, HID])
    x1l1_full = dint("x1l1_full", [EDGE_TAB, HID], True)
    x0p_loc = dint("x0p_loc", [NODE_PAD, HID])
    x0p_full = dint("x0p_full", [NODE_TAB, HID], True)
    x1l2_loc = dint("x1l2_loc", [EDGE_PAD, HID])
    x1l2_full = dint("x1l2_full", [EDGE_TAB, HID], True)
    armax_in = nc.dram_tensor("armax_in", [P, 1], dt.float32, kind="Internal")
    armax_out = nc.dram_tensor("armax_out", [P, 1], dt.float32,
                               kind="Internal", addr_space="Shared")

    UNROLL = int(os.environ.get("UNROLL", "2"))
    PH = int(os.environ.get("PHASES", "4"))
    rg = [list(range(NCORES))]

    with tile.TileContext(nc) as tc:
        with tc.tile_pool(name="const", bufs=1) as cp, \
             tc.tile_pool(name="meta", bufs=1) as mp, \
             tc.tile_pool(name="stg", bufs=2) as tp_stg, \
             tc.tile_pool(name="gt", bufs=2) as gp, \
             tc.tile_pool(name="st", bufs=3) as sp, \
             tc.tile_pool(name="fl", bufs=2) as fp, \
             tc.tile_pool(name="psw", bufs=2, space="PSUM") as pw, \
             tc.tile_pool(name="psm", bufs=2, space="PSUM") as pm, \
             tc.tile_pool(name="pst", bufs=2, space="PSUM") as pt_pool:

            iota_t = cp.tile([P, WIN], dt.float32)
            ident_t = cp.tile([P, P], dt.bfloat16)
            nc.sync.dma_start(iota_t[:], iota_in[:])
            nc.sync.dma_start(ident_t[:], ident_in[:])
            wts, bias = {}, {}
            for k, hnd in w_in.items():
                t = cp.tile(list(hnd.shape), dt.bfloat16, tag=k)
                nc.sync.dma_start(t[:], hnd[:])
                wts[k] = t
            for k, hnd in b_in.items():
                t = cp.tile([P, 1], dt.float32, tag=k)
                nc.sync.dma_start(t[:], hnd[:])
                bias[k] = t
            linw_t = cp.tile([P, 1], dt.float32)
            nc.sync.dma_start(linw_t[:], lin_w[:])
            linb_t = cp.tile([1, 1], dt.float32)
            nc.sync.dma_start(linb_t[:], lin_b[:])
            maxacc = cp.tile([P, WIN], dt.bfloat16)
            nc.vector.memset(maxacc[:], -1.0)

            # metadata: preload + convert seg->f32, wgt->f32
            def load_meta(idx_h, seg_h, wgt_h, nblk, tagp):
                idx_t = mp.tile([P, nblk], dt.int32, tag=f"{tagp}i")
                seg16 = mp.tile([P, nblk], dt.int16, tag=f"{tagp}s16")
                wgt16 = mp.tile([P, nblk], dt.bfloat16, tag=f"{tagp}w16")
                nc.sync.dma_start(idx_t[:], idx_h[:])
                nc.sync.dma_start(seg16[:], seg_h[:])
                nc.sync.dma_start(wgt16[:], wgt_h[:])
                seg_t = mp.tile([P, nblk], dt.float32, tag=f"{tagp}s")
                wgt_t = mp.tile([P, nblk], dt.float32, tag=f"{tagp}w")
                nc.vector.tensor_copy(seg_t[:], seg16[:])
                nc.vector.tensor_copy(wgt_t[:], wgt16[:])
                return idx_t, seg_t, wgt_t

            e_meta = load_meta(e_idx, e_seg, e_wgt, s_e.n_blocks, "e")
            n_meta = load_meta(n_idx, n_seg, n_wgt, s_n.n_blocks, "n")

            # x0 upload -> local table
            x0s = fp.tile([P, (NODE_PAD // P) * 16], dt.bfloat16, tag="x0s")
            nc.sync.dma_start(
                x0s[:].rearrange("p (q f) -> p q f", f=16),
                x0_in[:].rearrange("(q p) f -> p q f", p=P))
            nc.sync.dma_start(
                x0_loc[:].rearrange("(q p) f -> p q f", p=P),
                x0s[:].rearrange("p (q f) -> p q f", f=16))

            def emit_window(wi, blk0, wb, meta, kin, table, wkey, bkey,
                            out_loc, maxpool_nv=None):
                """Emit one window's work. wi/blk0 may be symbolic."""
                idx_t, seg_t, wgt_t = meta
                stg = tp_stg.tile([P, wb], dt.int32, tag=f"stg{wb}")
                nc.vector.tensor_copy(stg[:], idx_t[:, ds(blk0, wb)])
                gw = gp.tile([P, wb * kin], dt.bfloat16, tag=f"gw{kin}")
                for j in range(wb):
                    nc.gpsimd.indirect_dma_start(
                        out=gw[:, j * kin:(j + 1) * kin],
                        out_offset=None,
                        in_=table[:],
                        in_offset=IndirectOffsetOnAxis(ap=stg[:, j:j + 1],
                                                       axis=0),
                    )
                kdim = kin
                pt = pw.tile([kdim, WIN], dt.float32, tag=f"win{kdim}",
                             space="PSUM")
                for j in range(wb):
                    s_t = sp.tile([P, WIN], dt.bfloat16, tag="s")
                    nc.vector.tensor_scalar(
                        out=s_t[:], in0=iota_t[:],
                        scalar1=seg_t[:, ds(blk0 + j, 1)],
                        scalar2=wgt_t[:, ds(blk0 + j, 1)],
                        op0=mybir.AluOpType.is_equal,
                        op1=mybir.AluOpType.mult)
                    nc.tensor.matmul(pt[:], lhsT=gw[:, j * kin:(j + 1) * kin],
                                     rhs=s_t[:], start=(j == 0),
                                     stop=(j == wb - 1))
                aggt = fp.tile([kdim, WIN], dt.bfloat16, tag=f"aggt{kdim}")
                nc.vector.tensor_copy(aggt[:], pt[:])
                pmt = pm.tile([P, WIN], dt.float32, tag="m", space="PSUM")
                nc.tensor.matmul(pmt[:], lhsT=wts[wkey][:], rhs=aggt[:],
                                 start=True, stop=True)
                xt = fp.tile([P, WIN], dt.bfloat16, tag="xt")
                nc.scalar.activation(xt[:], pmt[:],
                                     mybir.ActivationFunctionType.Sigmoid,
                                     bias=bias[bkey][:, :1], scale=1.0)
                if maxpool_nv is not None:
                    nc.vector.tensor_tensor(
                        out=maxacc[:, :maxpool_nv], in0=maxacc[:, :maxpool_nv],
                        in1=xt[:, :maxpool_nv], op=mybir.AluOpType.max)
                else:
                    rowt = fp.tile([P, WIN], dt.bfloat16, tag="rowt")
                    for q in range(4):
                        pt2 = pt_pool.tile([P, P], dt.bfloat16, tag="tp",
                                           space="PSUM")
                        nc.tensor.transpose(pt2[:], xt[:, q * P:(q + 1) * P],
                                            ident_t[:])
                        nc.vector.tensor_copy(rowt[:, q * P:(q + 1) * P],
                                              pt2[:])
                    nc.sync.dma_start(
                        out_loc[ds(wi * WIN, WIN), :].rearrange(
                            "(p q) f -> p q f", q=4),
                        rowt[:].rearrange("p (q f) -> p q f", q=4))

            def run_agg(sched, meta, kin, table, wkey, bkey, out_loc,
                        maxpool=False):
                for (w0, w1, wb, base) in sched.ranges:
                    w1l = w1
                    if maxpool and w1 == sched.ranges[-1][1]:
                        w1l = w1 - 1          # peel last window
                    tc.For_i_unrolled(
                        w0, w1l, 1,
                        lambda wi, _w0=w0, _wb=wb, _base=base: emit_window(
                            wi, _base + (wi - _w0) * _wb, _wb, meta, kin,
                            table, wkey, bkey, out_loc,
                            maxpool_nv=WIN if maxpool else None),
                        max_unroll=UNROLL)
                if maxpool:
                    w0, w1, wb, base = sched.ranges[-1]
                    nv = NODE_LOC - (w1 - 1) * WIN
                    emit_window(w1 - 1, base + (w1 - 1 - w0) * wb, wb, meta,
                                kin, table, wkey, bkey, out_loc,
                                maxpool_nv=nv)

            def allgather(src, dst):
                nc.gpsimd.collective_compute(
                    "AllGather", mybir.AluOpType.bypass, replica_groups=rg,
                    ins=[src[:]], outs=[dst[:]])

            allgather(x0_loc, x0_full)
            run_agg(s_e, e_meta, 16, x0_full, "w0_1", "b1_1", x1l1_loc)
            if PH >= 2:
                allgather(x1l1_loc, x1l1_full)
                run_agg(s_n, n_meta, HID, x1l1_full, "w1_1", "b0_1", x0p_loc)
            if PH >= 3:
                allgather(x0p_loc, x0p_full)
                run_agg(s_e, e_meta, HID, x0p_full, "w0_2", "b1_2", x1l2_loc)
            if PH >= 4:
                allgather(x1l2_loc, x1l2_full)
                run_agg(s_n, n_meta, HID, x1l2_full, "w1_2", "b0_2", None,
                        maxpool=True)

            mx = fp.tile([P, 1], dt.float32, tag="mx")
            nc.vector.reduce_max(out=mx[:], in_=maxacc[:],
                                 axis=mybir.AxisListType.X)
            nc.sync.dma_start(armax_in[:], mx[:])
            nc.gpsimd.collective_compute(
                "AllReduce", mybir.AluOpType.max, replica_groups=rg,
                ins=[armax_in[:]], outs=[armax_out[:]])
            mx2 = fp.tile([P, 1], dt.float32, tag="mx2")
            nc.sync.dma_start(mx2[:], armax_out[:])
            prod = fp.tile([P, 1], dt.float32, tag="prod")
            nc.vector.tensor_mul(prod[:], mx2[:], linw_t[:])
            ones = cp.tile([P, 1], dt.float32, tag="ones")
            nc.vector.memset(ones[:], 1.0)
            psf = pm.tile([1, 1], dt.float32, tag="m", space="PSUM")
            nc.tensor.matmul(psf[:], lhsT=prod[:], rhs=ones[:],
                             start=True, stop=True)
            res = fp.tile([1, 1], dt.float32, tag="res")
            nc.scalar.activation(res[:], psf[:],
                                 mybir.ActivationFunctionType.Identity,
                                 bias=linb_t[:, :1], scale=1.0)
            nc.sync.dma_start(out_t[:], res[:])

    nc.compile()
    return nc


def make_in_maps(pre, inputs):
    s_e, s_n = pre["sched_e"], pre["sched_n"]
    iota = np.broadcast_to(np.arange(WIN, dtype=f32), (P, WIN)).copy()
    ident = np.eye(P, dtype=bf16)

    def b_t(x):
        return np.asarray(x).astype(f32).reshape(HID, 1)

    w0_1 = np.zeros((16, HID), bf16)
    w0_1[:IN_CH] = np.asarray(inputs["w0_l1"]).astype(bf16)
    in_maps = []
    for r in range(NCORES):
        in_maps.append(dict(
            x0_in=pre["x0tab"][r],
            e_idx=np.ascontiguousarray(s_e.idx[r]),
            e_seg=np.ascontiguousarray(s_e.seg[r]),
            e_wgt=np.ascontiguousarray(s_e.wgt[r]),
            n_idx=np.ascontiguousarray(s_n.idx[r]),
            n_seg=np.ascontiguousarray(s_n.seg[r]),
            n_wgt=np.ascontiguousarray(s_n.wgt[r]),
            w0_1=w0_1,
            w1_1=np.asarray(inputs["w1_l1"]).astype(bf16),
            w0_2=np.asarray(inputs["w0_l2"]).astype(bf16),
            w1_2=np.asarray(inputs["w1_l2"]).astype(bf16),
            b1_1=b_t(inputs["b1_l1"]), b0_1=b_t(inputs["b0_l1"]),
            b1_2=b_t(inputs["b1_l2"]), b0_2=b_t(inputs["b0_l2"]),
            lin_w=np.asarray(inputs["lin_w"]).astype(f32).reshape(HID, 1),
            lin_b=np.asarray(inputs["lin_b"]).astype(f32).reshape(1, 1),
            iota=iota, ident=ident,
        ))
    return in_maps


def kernel(**inputs):
    pre = _preprocess(inputs)
    nc = _build(pre)
    in_maps = make_in_maps(pre, inputs)
    from concourse.bass_utils import run_bass_kernel_spmd
    res = run_bass_kernel_spmd(nc, in_maps, core_ids=list(range(NCORES)))
    out = res.results[0]["out"].reshape(1).astype(f32)
    return out
